# revision 6
# baseline (speedup 1.0000x reference)
"""Capsule-routing kernel v3 for Trainium2 (8 NeuronCores, data-parallel).

Math (u_hat never materialized):
  iter1: c uniform=0.1 -> o1 = 0.1*(sum_n u) @ W_j    (host-precomputed -> q1)
  iter t: Q[:,j] = W_j @ o[j]; logits b = u @ Q; c = softmax_j(b)
          R.T[d,j] = sum_n u[n,d] c[n,j];  o[j] = R[j,:] @ W_j
  out = squash(o3)  (host epilogue)

v3 vs baseline:
  - u loaded ONCE per layout in fp16 (10-bit mantissa): ut [d,n] for logits
    (stationary, FWL), un [n,d] for R (stationary). 8.4MB/core vs 17MB.
  - logits moving operand q is f32r (mixed-dtype MM) or fp16 hi/lo pair.
  - iter-1 chain (depends only on row sums of u) computed on host -> q1.
  - samples processed in pairs to amortize DVE/ACT fixed costs; softmax
    reads logits PSUM directly (no copy).
  - emission is phase-blocked across pairs so PE never waits on DVE.
"""

import os
import sys

import numpy as np

for _p in ("/opt/trn_rl_repo", "/opt/trn_rl_repo/concourse"):
    if _p not in sys.path and os.path.isdir(_p):
        sys.path.insert(0, _p)

import concourse.bass as bass
import concourse.mybir as mybir
import concourse.tile as tile
from concourse import bacc

F32 = mybir.dt.float32
F32R = mybir.dt.float32r
F16 = mybir.dt.float16
AF = mybir.ActivationFunctionType
AX = mybir.AxisListType
ALU = mybir.AluOpType

N_CORES = 8
B_FULL, N, D = 64, 2048, 128
J, DC = 10, 16
JD = J * DC          # 160
NT = N // 128        # 16 chunks of n per sample
B_LOC = B_FULL // N_CORES  # 8 samples per core
NP = B_LOC // 2      # 4 sample pairs
EPS = 1e-7

Q_MODE = os.environ.get("CAPS_Q_MODE", "single")  # "single" (f16 q) | "hilo" (f16 q pair)
WARMUP_MMS = int(os.environ.get("CAPS_WARMUP", "16"))


def _bcast(ap, extra):
    """Append step-0 (broadcast) dims to an AP."""
    return bass.AP(tensor=ap.tensor, offset=ap.offset,
                   ap=list(ap.ap) + [[0, n] for n in extra])


def _bcast_mid(ap, idx, n):
    """Insert a step-0 (broadcast) dim of extent n at position idx (free dims
    count partition as 0)."""
    aps = list(ap.ap)
    aps.insert(idx, [0, n])
    return bass.AP(tensor=ap.tensor, offset=ap.offset, ap=aps)


def build_program(for_sim=False):
    if for_sim:
        nc = bacc.Bacc(None, target_bir_lowering=False, debug=True)
    else:
        nc = bacc.Bacc(None)

    QW = 10 if Q_MODE == "single" else 20
    QDT = F16

    ut_d = nc.declare_dram_parameter("ut", [B_LOC, D, N], F16, isOutput=False)
    un_d = nc.declare_dram_parameter("un", [B_LOC, D, NT, D], F16, isOutput=False)
    q1_d = nc.declare_dram_parameter("q1", [D, B_LOC, QW], QDT, isOutput=False)
    w_d = nc.declare_dram_parameter("w", [D, JD], F32, isOutput=False)
    om_d = nc.declare_dram_parameter("ones_mat", [D, D], F32, isOutput=False)
    om16_d = nc.declare_dram_parameter("ones16", [D, D], F16, isOutput=False)
    out_d = nc.declare_dram_parameter("out", [1, B_LOC * JD], F32, isOutput=True)

    with tile.TileContext(nc) as tc:
        with (
            tc.tile_pool(name="big", bufs=1) as big,
            tc.tile_pool(name="consts", bufs=1) as consts,
            tc.tile_pool(name="sm", bufs=3) as sm,
            tc.tile_pool(name="chain", bufs=3) as chain,
            tc.tile_pool(name="psumB", bufs=3, space="PSUM") as psumB,
            tc.tile_pool(name="psumR", bufs=2, space="PSUM") as psumR,
            tc.tile_pool(name="psumC", bufs=2, space="PSUM") as psumC,
            tc.tile_pool(name="psumW", bufs=1, space="PSUM") as psumW,
        ):
            w_sb = consts.tile([D, JD], F32)
            ones_r = consts.tile([D, D], F32)
            q1_sb = consts.tile([D, B_LOC, QW], QDT)
            out_sb = consts.tile([1, B_LOC * JD], F32)
            ones16 = consts.tile([D, D], F16)
            # early tiny consts on the gpsimd ring; mid-kernel consts on
            # scalar; sync carries only the big streams.
            nc.gpsimd.dma_start(out=ones16[:], in_=om16_d[:])
            nc.gpsimd.dma_start(out=q1_sb[:], in_=q1_d[:])
            nc.scalar.dma_start(out=w_sb[:], in_=w_d[:])
            nc.scalar.dma_start(out=ones_r[:], in_=om_d[:])

            w_jd = w_sb[:].rearrange("p (j d) -> p j d", j=J)

            ut = [big.tile([D, NT, D], F16, tag=f"ut{b}", name=f"ut{b}")
                  for b in range(B_LOC)]
            un = [big.tile([D, NT, D], F16, tag=f"un{b}", name=f"un{b}")
                  for b in range(B_LOC)]

            # HAM warmup: dense junk matmuls while the first DMAs land.
            junk = psumW.tile([D, 32], F32, tag="junk", name="junk")
            for _ in range(WARMUP_MMS):
                nc.tensor.matmul(junk[:], ones16[:], ones16[:, 0:32],
                                 start=True, stop=True)

            # big streams on sync, ordered by first consumer
            big_order = ["ut0", "ut1", "un0", "ut2", "un1", "ut3", "un2",
                         "ut4", "un3", "ut5", "un4", "ut6", "un5", "ut7",
                         "un6", "un7"]
            for name in big_order:
                b = int(name[2])
                if name.startswith("ut"):
                    nc.sync.dma_start(
                        out=ut[b][:],
                        in_=ut_d[b, :, :].rearrange("p (t n) -> p t n", t=NT))
                else:
                    nc.sync.dma_start(out=un[b][:], in_=un_d[b])

            def logits(p, q_aps):
                """q_aps: per-sample [D, QW] moving APs.
                hilo: two per-sample [D, NT, 2J] PSUM tiles (hi|lo cols).
                single: one [D, 2, NT, J] pair PSUM tile."""
                if Q_MODE == "single":
                    bp = psumB.tile([D, 2, NT, J], F32, tag="bp")
                    for s in range(2):
                        b = 2 * p + s
                        for t in range(NT):
                            nc.tensor.matmul(bp[:, s, t, :], ut[b][:, t, :],
                                             q_aps[s], start=True, stop=True)
                    return bp
                bps = []
                for s in range(2):
                    b = 2 * p + s
                    bp = psumB.tile([D, NT, 2 * J], F32, tag="bp")
                    for t in range(NT):
                        nc.tensor.matmul(bp[:, t, :], ut[b][:, t, :],
                                         q_aps[s], start=True, stop=True)
                    bps.append(bp)
                return bps

            def softmax(bp):
                """-> c [D, 2, NT, J] fp16."""
                if Q_MODE == "single":
                    bsum = bp[:]           # PSUM AP [D, 2, NT, J]
                    negm = sm.tile([D, 2, NT], F32, tag="negm")
                    nc.vector.reduce_max(negm[:], bsum, axis=AX.X, negate=True)
                else:
                    # ACT-copy each sample's hi|lo PSUM to SBUF, fold on DVE
                    bsum_t = sm.tile([D, 2, NT, J], F32, tag="bsum")
                    for s in range(2):
                        bpc = sm.tile([D, NT, 2 * J], F32, tag=f"bpc{s}")
                        nc.scalar.activation(
                            bpc[:].rearrange("p t j -> p (t j)"),
                            bp[s][:].rearrange("p t j -> p (t j)"), AF.Copy)
                        nc.vector.tensor_add(bsum_t[:, s], bpc[:, :, 0:J],
                                             bpc[:, :, J:2 * J])
                    bsum = bsum_t[:]
                    negm = sm.tile([D, 2, NT], F32, tag="negm")
                    nc.vector.reduce_max(negm[:], bsum, axis=AX.X, negate=True)
                bs = sm.tile([D, 2, NT, J], F32, tag="bs")
                nc.vector.tensor_add(bs[:], bsum, _bcast(negm[:], [J]))
                e = sm.tile([D, 2, NT, J], F16, tag="e")
                nc.scalar.activation(
                    e[:].rearrange("p a t j -> p (a t j)"),
                    bs[:].rearrange("p a t j -> p (a t j)"), AF.Exp)
                z = sm.tile([D, 2, NT], F32, tag="z")
                with nc.allow_low_precision(reason="z sums 10 fp16 probs"):
                    nc.vector.reduce_sum(z[:], e[:], axis=AX.X)
                zr = sm.tile([D, 2, NT], F16, tag="zr")
                with nc.allow_low_precision(reason="zr fp16; z in [1,10]"):
                    nc.vector.reciprocal(zr[:], z[:])
                c = sm.tile([D, 2, NT, J], F16, tag="c")
                nc.vector.tensor_mul(c[:], e[:], _bcast(zr[:], [J]))
                return c

            def r_pass(p, c):
                rt = psumR.tile([D, 2, J], F32, tag="rt")
                for s in range(2):
                    b = 2 * p + s
                    for t in range(NT):
                        nc.tensor.matmul(rt[:, s, :], un[b][:, t, :],
                                         c[:, s, t, :], start=(t == 0),
                                         stop=(t == NT - 1))
                return rt

            def ochain(p, rt, is_last):
                """rt: [D, 2, J] PSUM (R.T per sample). -> q pair or None.
                PSUM->SBUF hops on ACT; elementwise/reduce on GPSIMD (DVE is
                the softmax bottleneck, GPSIMD is otherwise idle)."""
                rt_sb = chain.tile([D, 2, J], F32, tag="rts")
                nc.scalar.activation(
                    rt_sb[:].rearrange("p a j -> p (a j)"),
                    rt[:].rearrange("p a j -> p (a j)"), AF.Copy)
                m1 = chain.tile([D, 2, J, DC], F32, tag="m1")
                nc.gpsimd.tensor_mul(m1[:], _bcast_mid(w_jd, 1, 2),
                                     _bcast(rt_sb[:], [DC]))
                obc = psumC.tile([D, 2, JD], F32, tag="obc")
                nc.tensor.matmul(obc[:].rearrange("p a f -> p (a f)"),
                                 ones_r[:],
                                 m1[:].rearrange("p a j d -> p (a j d)"),
                                 start=True, stop=True)
                if is_last:
                    nc.scalar.activation(
                        out_sb[0:1, 2 * p * JD:(2 * p + 2) * JD],
                        obc[0:1, :, :].rearrange("p a f -> p (a f)"), AF.Copy)
                    return None
                obc_sb = chain.tile([D, 2, JD], F32, tag="obcs")
                nc.scalar.activation(
                    obc_sb[:].rearrange("p a f -> p (a f)"),
                    obc[:].rearrange("p a f -> p (a f)"), AF.Copy)
                qw = chain.tile([D, 2, J, DC], F32, tag="qw")
                nc.gpsimd.tensor_mul(
                    qw[:], _bcast_mid(w_jd, 1, 2),
                    obc_sb[:].rearrange("p a (j d) -> p a j d", j=J))
                q = chain.tile([D, 2, J], F16, tag="q")
                with nc.allow_low_precision(reason="q fp16 feeds fp16 MM"):
                    nc.vector.reduce_sum(q[:], qw[:], axis=AX.X)
                return [q[:, 0, :], q[:, 1, :]]

            # Interleaved emission: PE executes in emission order, so order
            # blocks by data arrival (ut0..ut7 then un0..un7) and keep
            # un7-dependent work late while independent iter-3 work fills in.
            ORDER = ["L2_0", "R2_0", "L2_1", "R2_1", "O2_0", "L3_0",
                     "L2_2", "O2_1", "R2_2", "R3_0", "L3_1", "L2_3",
                     "O2_2", "O3_0", "R2_3", "L3_2", "R3_1", "O2_3",
                     "O3_1", "L3_3", "R3_2", "O3_2", "R3_3", "O3_3"]
            q_cur = {p: [q1_sb[:, 2 * p, :], q1_sb[:, 2 * p + 1, :]]
                     for p in range(NP)}
            cs, rts = {}, {}
            for blk in ORDER:
                kind, p = blk.split("_")
                p = int(p)
                if kind in ("L2", "L3"):
                    cs[p] = softmax(logits(p, q_cur[p]))
                elif kind in ("R2", "R3"):
                    rts[p] = r_pass(p, cs[p])
                elif kind == "O2":
                    q_cur[p] = ochain(p, rts[p], False)
                else:
                    ochain(p, rts[p], True)

            nc.sync.dma_start(out=out_d[:], in_=out_sb[:])

    nc.compile()
    return nc


def _f32r(x):
    xi = np.ascontiguousarray(x, np.float32).view(np.uint32).astype(np.int64)
    bias = ((xi >> 12) & 1) + (1 << 11) - 1
    return (((xi + bias) >> 12) << 12).astype(np.uint32).view(np.float32)


def _squash(o):
    s2 = (o ** 2).sum(-1, keepdims=True)
    return o * s2 / ((1.0 + s2) * np.sqrt(s2 + EPS))


def host_inputs(u_core, W):
    """Per-core host prep: u_core [B_LOC, N, D] f32, W [D, JD] f32."""
    us = np.ascontiguousarray(u_core, np.float32)
    ut = np.ascontiguousarray(us.transpose(0, 2, 1)).astype(np.float16)
    un = np.ascontiguousarray(
        us.reshape(B_LOC, NT, D, D).transpose(0, 2, 1, 3)).astype(np.float16)
    # iter-1 chain on host: r1 = 0.1*sum_n u -> o1 -> q1
    Wr = W.reshape(D, J, DC)
    r1 = 0.1 * us.sum(axis=1)                         # [B_LOC, D]
    m1 = _f32r(Wr[None] * r1[:, :, None, None])       # [B_LOC, D, J, DC]
    o1 = m1.sum(axis=1)                               # [B_LOC, J, DC]
    q1 = (Wr[None] * o1[:, None, :, :]).sum(-1)       # [B_LOC, D, J]
    if Q_MODE == "single":
        q1_np = np.ascontiguousarray(q1.astype(np.float16).transpose(1, 0, 2))
    else:
        q1h = q1.astype(np.float16)
        q1l = (q1 - q1h.astype(np.float32)).astype(np.float16)
        q1_np = np.ascontiguousarray(
            np.concatenate([q1h, q1l], axis=-1).transpose(1, 0, 2))
    return {
        "ut": ut,
        "un": un,
        "q1": q1_np,
        "w": np.ascontiguousarray(W, np.float32),
        "ones_mat": np.ones((D, D), np.float32),
        "ones16": np.ones((D, D), np.float16),
    }


_NC = None


def _get_nc():
    global _NC
    if _NC is None:
        _NC = build_program()
    return _NC


def run_sharded(u_vecs: np.ndarray, W: np.ndarray, **kw):
    """Shard over 8 cores, run, return (full_output, BassKernelResults)."""
    from concourse.bass_utils import run_bass_kernel_spmd

    u_vecs = np.ascontiguousarray(u_vecs, dtype=np.float32)
    W = np.ascontiguousarray(W, dtype=np.float32)
    assert u_vecs.shape == (B_FULL, N, D) and W.shape == (D, JD)

    nc = _get_nc()
    in_maps = [host_inputs(u_vecs[k * B_LOC:(k + 1) * B_LOC], W)
               for k in range(N_CORES)]
    res = run_bass_kernel_spmd(nc, in_maps, core_ids=list(range(N_CORES)), **kw)
    o3 = np.concatenate(
        [res.results[k]["out"].reshape(B_LOC, JD) for k in range(N_CORES)],
        axis=0)
    out = _squash(o3.reshape(B_FULL, J, DC).astype(np.float32))
    return out.astype(np.float32), res


def kernel(u_vecs: np.ndarray, W: np.ndarray) -> np.ndarray:
    out, _ = run_sharded(u_vecs, W)
    return out


# revision 7
# speedup vs baseline: 1.0368x; 1.0368x over previous
"""Capsule-routing kernel v3 for Trainium2 (8 NeuronCores, data-parallel).

Math (u_hat never materialized):
  iter1: c uniform=0.1 -> o1 = 0.1*(sum_n u) @ W_j    (host-precomputed -> q1)
  iter t: Q[:,j] = W_j @ o[j]; logits b = u @ Q; c = softmax_j(b)
          R.T[d,j] = sum_n u[n,d] c[n,j];  o[j] = R[j,:] @ W_j
  out = squash(o3)  (host epilogue)

v3 vs baseline:
  - u loaded ONCE per layout in fp16 (10-bit mantissa): ut [d,n] for logits
    (stationary, FWL), un [n,d] for R (stationary). 8.4MB/core vs 17MB.
  - logits moving operand q is f32r (mixed-dtype MM) or fp16 hi/lo pair.
  - iter-1 chain (depends only on row sums of u) computed on host -> q1.
  - samples processed in pairs to amortize DVE/ACT fixed costs; softmax
    reads logits PSUM directly (no copy).
  - emission is phase-blocked across pairs so PE never waits on DVE.
"""

import os
import sys

import numpy as np

for _p in ("/opt/trn_rl_repo", "/opt/trn_rl_repo/concourse"):
    if _p not in sys.path and os.path.isdir(_p):
        sys.path.insert(0, _p)

import concourse.bass as bass
import concourse.mybir as mybir
import concourse.tile as tile
from concourse import bacc

F32 = mybir.dt.float32
F32R = mybir.dt.float32r
F16 = mybir.dt.float16
AF = mybir.ActivationFunctionType
AX = mybir.AxisListType
ALU = mybir.AluOpType

N_CORES = 8
B_FULL, N, D = 64, 2048, 128
J, DC = 10, 16
JD = J * DC          # 160
NT = N // 128        # 16 chunks of n per sample
B_LOC = B_FULL // N_CORES  # 8 samples per core
NP = B_LOC // 2      # 4 sample pairs
EPS = 1e-7

Q_MODE = os.environ.get("CAPS_Q_MODE", "single")  # "single" (f16 q) | "hilo" (f16 q pair)
WARMUP_MMS = int(os.environ.get("CAPS_WARMUP", "16"))


def _bcast(ap, extra):
    """Append step-0 (broadcast) dims to an AP."""
    return bass.AP(tensor=ap.tensor, offset=ap.offset,
                   ap=list(ap.ap) + [[0, n] for n in extra])


def _bcast_mid(ap, idx, n):
    """Insert a step-0 (broadcast) dim of extent n at position idx (free dims
    count partition as 0)."""
    aps = list(ap.ap)
    aps.insert(idx, [0, n])
    return bass.AP(tensor=ap.tensor, offset=ap.offset, ap=aps)


def build_program(for_sim=False):
    if for_sim:
        nc = bacc.Bacc(None, target_bir_lowering=False, debug=True)
    else:
        nc = bacc.Bacc(None)

    QW = 10 if Q_MODE == "single" else 20
    QDT = F16

    ut_d = nc.declare_dram_parameter("ut", [B_LOC, D, N], F16, isOutput=False)
    un_d = nc.declare_dram_parameter("un", [B_LOC, D, NT, D], F16, isOutput=False)
    q1_d = nc.declare_dram_parameter("q1", [D, B_LOC, QW], QDT, isOutput=False)
    w_d = nc.declare_dram_parameter("w", [D, JD], F32, isOutput=False)
    om_d = nc.declare_dram_parameter("ones_mat", [D, D], F32R, isOutput=False)
    om16_d = nc.declare_dram_parameter("ones16", [D, D], F16, isOutput=False)
    out_d = nc.declare_dram_parameter("out", [1, B_LOC * JD], F32, isOutput=True)

    with tile.TileContext(nc) as tc:
        with (
            tc.tile_pool(name="big", bufs=1) as big,
            tc.tile_pool(name="consts", bufs=1) as consts,
            tc.tile_pool(name="sm", bufs=3) as sm,
            tc.tile_pool(name="chain", bufs=3) as chain,
            tc.tile_pool(name="psumB", bufs=3, space="PSUM") as psumB,
            tc.tile_pool(name="psumR", bufs=2, space="PSUM") as psumR,
            tc.tile_pool(name="psumC", bufs=2, space="PSUM") as psumC,
            tc.tile_pool(name="psumW", bufs=1, space="PSUM") as psumW,
        ):
            w_sb = consts.tile([D, JD], F32)
            ones_r = consts.tile([D, D], F32R)
            q1_sb = consts.tile([D, B_LOC, QW], QDT)
            out_sb = consts.tile([1, B_LOC * JD], F32)
            ones16 = consts.tile([D, D], F16)
            # early tiny consts on the gpsimd ring; mid-kernel consts on
            # scalar; sync carries only the big streams.
            nc.gpsimd.dma_start(out=ones16[:], in_=om16_d[:])
            nc.gpsimd.dma_start(out=q1_sb[:], in_=q1_d[:])
            nc.scalar.dma_start(out=w_sb[:], in_=w_d[:])
            nc.scalar.dma_start(out=ones_r[:], in_=om_d[:])

            w_jd = w_sb[:].rearrange("p (j d) -> p j d", j=J)

            ut = [big.tile([D, NT, D], F16, tag=f"ut{b}", name=f"ut{b}")
                  for b in range(B_LOC)]
            un = [big.tile([D, NT, D], F16, tag=f"un{b}", name=f"un{b}")
                  for b in range(B_LOC)]

            # HAM warmup: dense junk matmuls while the first DMAs land.
            junk = psumW.tile([D, 32], F32, tag="junk", name="junk")
            for _ in range(WARMUP_MMS):
                nc.tensor.matmul(junk[:], ones16[:], ones16[:, 0:32],
                                 start=True, stop=True)

            # big streams on sync, ordered by first consumer
            big_order = ["ut0", "ut1", "un0", "ut2", "un1", "ut3", "un2",
                         "ut4", "un3", "ut5", "un4", "ut6", "un5", "ut7",
                         "un6", "un7"]
            for name in big_order:
                b = int(name[2])
                if name.startswith("ut"):
                    nc.sync.dma_start(
                        out=ut[b][:],
                        in_=ut_d[b, :, :].rearrange("p (t n) -> p t n", t=NT))
                else:
                    nc.sync.dma_start(out=un[b][:], in_=un_d[b])

            def logits(p, q_aps):
                """q_aps: per-sample [D, QW] moving APs.
                hilo: two per-sample [D, NT, 2J] PSUM tiles (hi|lo cols).
                single: one [D, 2, NT, J] pair PSUM tile."""
                if Q_MODE == "single":
                    bp = psumB.tile([D, 2, NT, J], F32, tag="bp")
                    for s in range(2):
                        b = 2 * p + s
                        for t in range(NT):
                            nc.tensor.matmul(bp[:, s, t, :], ut[b][:, t, :],
                                             q_aps[s], start=True, stop=True)
                    return bp
                bps = []
                for s in range(2):
                    b = 2 * p + s
                    bp = psumB.tile([D, NT, 2 * J], F32, tag="bp")
                    for t in range(NT):
                        nc.tensor.matmul(bp[:, t, :], ut[b][:, t, :],
                                         q_aps[s], start=True, stop=True)
                    bps.append(bp)
                return bps

            def softmax(bp):
                """-> c [D, 2, NT, J] fp16."""
                if Q_MODE == "single":
                    bsum = bp[:]           # PSUM AP [D, 2, NT, J]
                    negm = sm.tile([D, 2, NT], F32, tag="negm")
                    nc.vector.reduce_max(negm[:], bsum, axis=AX.X, negate=True)
                else:
                    # ACT-copy each sample's hi|lo PSUM to SBUF, fold on DVE
                    bsum_t = sm.tile([D, 2, NT, J], F32, tag="bsum")
                    for s in range(2):
                        bpc = sm.tile([D, NT, 2 * J], F32, tag=f"bpc{s}")
                        nc.scalar.activation(
                            bpc[:].rearrange("p t j -> p (t j)"),
                            bp[s][:].rearrange("p t j -> p (t j)"), AF.Copy)
                        nc.vector.tensor_add(bsum_t[:, s], bpc[:, :, 0:J],
                                             bpc[:, :, J:2 * J])
                    bsum = bsum_t[:]
                    negm = sm.tile([D, 2, NT], F32, tag="negm")
                    nc.vector.reduce_max(negm[:], bsum, axis=AX.X, negate=True)
                bs = sm.tile([D, 2, NT, J], F32, tag="bs")
                nc.vector.tensor_add(bs[:], bsum, _bcast(negm[:], [J]))
                e = sm.tile([D, 2, NT, J], F16, tag="e")
                nc.scalar.activation(
                    e[:].rearrange("p a t j -> p (a t j)"),
                    bs[:].rearrange("p a t j -> p (a t j)"), AF.Exp)
                z = sm.tile([D, 2, NT], F32, tag="z")
                with nc.allow_low_precision(reason="z sums 10 fp16 probs"):
                    nc.vector.reduce_sum(z[:], e[:], axis=AX.X)
                zr = sm.tile([D, 2, NT], F16, tag="zr")
                with nc.allow_low_precision(reason="zr fp16; z in [1,10]"):
                    nc.vector.reciprocal(zr[:], z[:])
                c = sm.tile([D, 2, NT, J], F16, tag="c")
                nc.vector.tensor_mul(c[:], e[:], _bcast(zr[:], [J]))
                return c

            def r_pass(p, c):
                rt = psumR.tile([D, 2, J], F32, tag="rt")
                for s in range(2):
                    b = 2 * p + s
                    for t in range(NT):
                        nc.tensor.matmul(rt[:, s, :], un[b][:, t, :],
                                         c[:, s, t, :], start=(t == 0),
                                         stop=(t == NT - 1))
                return rt

            def ochain(p, rt, is_last):
                """rt: [D, 2, J] PSUM (R.T per sample). -> q pair or None.
                PSUM->SBUF hops on ACT; elementwise/reduce on GPSIMD (DVE is
                the softmax bottleneck, GPSIMD is otherwise idle)."""
                m1 = chain.tile([D, 2, J, DC], F32R, tag="m1")
                nc.vector.tensor_mul(m1[:], _bcast_mid(w_jd, 1, 2),
                                     _bcast(rt[:], [DC]))
                obc = psumC.tile([D, 2, JD], F32, tag="obc")
                nc.tensor.matmul(obc[:].rearrange("p a f -> p (a f)"),
                                 ones_r[:],
                                 m1[:].rearrange("p a j d -> p (a j d)"),
                                 start=True, stop=True)
                if is_last:
                    nc.scalar.activation(
                        out_sb[0:1, 2 * p * JD:(2 * p + 2) * JD],
                        obc[0:1, :, :].rearrange("p a f -> p (a f)"), AF.Copy)
                    return None
                qw = chain.tile([D, 2, J, DC], F32, tag="qw")
                nc.vector.tensor_mul(
                    qw[:], _bcast_mid(w_jd, 1, 2),
                    obc[:].rearrange("p a (j d) -> p a j d", j=J))
                q = chain.tile([D, 2, J], F16, tag="q")
                with nc.allow_low_precision(reason="q fp16 feeds fp16 MM"):
                    nc.vector.reduce_sum(q[:], qw[:], axis=AX.X)
                return [q[:, 0, :], q[:, 1, :]]

            # Interleaved emission: PE executes in emission order, so order
            # blocks by data arrival (ut0..ut7 then un0..un7) and keep
            # un7-dependent work late while independent iter-3 work fills in.
            ORDER = ["L2_0", "R2_0", "L2_1", "R2_1", "O2_0", "L3_0",
                     "L2_2", "O2_1", "R2_2", "R3_0", "L3_1", "L2_3",
                     "O2_2", "O3_0", "R2_3", "L3_2", "R3_1", "O2_3",
                     "O3_1", "L3_3", "R3_2", "O3_2", "R3_3", "O3_3"]
            q_cur = {p: [q1_sb[:, 2 * p, :], q1_sb[:, 2 * p + 1, :]]
                     for p in range(NP)}
            FILL = {"R2_0": 16, "L2_1": 8, "R2_1": 10, "L3_0": 16,
                    "L2_2": 8, "R2_2": 18, "L3_1": 8, "L2_3": 12,
                    "R2_3": 18, "L3_2": 8, "R3_1": 8, "L3_3": 8,
                    "R3_2": 8, "R3_3": 8}
            fscale = float(os.environ.get("CAPS_FILL", "1.0"))
            cs, rts = {}, {}
            for blk in ORDER:
                kind, p = blk.split("_")
                p = int(p)
                for _ in range(int(FILL.get(blk, 0) * fscale)):
                    nc.tensor.matmul(junk[:], ones16[:], ones16[:, 0:32],
                                     start=True, stop=True)
                if kind in ("L2", "L3"):
                    cs[p] = softmax(logits(p, q_cur[p]))
                elif kind in ("R2", "R3"):
                    rts[p] = r_pass(p, cs[p])
                elif kind == "O2":
                    q_cur[p] = ochain(p, rts[p], False)
                else:
                    ochain(p, rts[p], True)

            nc.sync.dma_start(out=out_d[:], in_=out_sb[:])

    nc.compile()
    return nc


def _f32r(x):
    xi = np.ascontiguousarray(x, np.float32).view(np.uint32).astype(np.int64)
    bias = ((xi >> 12) & 1) + (1 << 11) - 1
    return (((xi + bias) >> 12) << 12).astype(np.uint32).view(np.float32)


def _squash(o):
    s2 = (o ** 2).sum(-1, keepdims=True)
    return o * s2 / ((1.0 + s2) * np.sqrt(s2 + EPS))


def host_inputs(u_core, W):
    """Per-core host prep: u_core [B_LOC, N, D] f32, W [D, JD] f32."""
    us = np.ascontiguousarray(u_core, np.float32)
    ut = np.ascontiguousarray(us.transpose(0, 2, 1)).astype(np.float16)
    un = np.ascontiguousarray(
        us.reshape(B_LOC, NT, D, D).transpose(0, 2, 1, 3)).astype(np.float16)
    # iter-1 chain on host: r1 = 0.1*sum_n u -> o1 -> q1
    Wr = W.reshape(D, J, DC)
    r1 = 0.1 * us.sum(axis=1)                         # [B_LOC, D]
    m1 = _f32r(Wr[None] * r1[:, :, None, None])       # [B_LOC, D, J, DC]
    o1 = m1.sum(axis=1)                               # [B_LOC, J, DC]
    q1 = (Wr[None] * o1[:, None, :, :]).sum(-1)       # [B_LOC, D, J]
    if Q_MODE == "single":
        q1_np = np.ascontiguousarray(q1.astype(np.float16).transpose(1, 0, 2))
    else:
        q1h = q1.astype(np.float16)
        q1l = (q1 - q1h.astype(np.float32)).astype(np.float16)
        q1_np = np.ascontiguousarray(
            np.concatenate([q1h, q1l], axis=-1).transpose(1, 0, 2))
    return {
        "ut": ut,
        "un": un,
        "q1": q1_np,
        "w": np.ascontiguousarray(W, np.float32),
        "ones_mat": np.ones((D, D), np.float32),
        "ones16": np.ones((D, D), np.float16),
    }


_NC = None


def _get_nc():
    global _NC
    if _NC is None:
        _NC = build_program()
    return _NC


def run_sharded(u_vecs: np.ndarray, W: np.ndarray, **kw):
    """Shard over 8 cores, run, return (full_output, BassKernelResults)."""
    from concourse.bass_utils import run_bass_kernel_spmd

    u_vecs = np.ascontiguousarray(u_vecs, dtype=np.float32)
    W = np.ascontiguousarray(W, dtype=np.float32)
    assert u_vecs.shape == (B_FULL, N, D) and W.shape == (D, JD)

    nc = _get_nc()
    in_maps = [host_inputs(u_vecs[k * B_LOC:(k + 1) * B_LOC], W)
               for k in range(N_CORES)]
    res = run_bass_kernel_spmd(nc, in_maps, core_ids=list(range(N_CORES)), **kw)
    o3 = np.concatenate(
        [res.results[k]["out"].reshape(B_LOC, JD) for k in range(N_CORES)],
        axis=0)
    out = _squash(o3.reshape(B_FULL, J, DC).astype(np.float32))
    return out.astype(np.float32), res


def kernel(u_vecs: np.ndarray, W: np.ndarray) -> np.ndarray:
    out, _ = run_sharded(u_vecs, W)
    return out


# revision 8
# speedup vs baseline: 1.1003x; 1.0613x over previous
"""Capsule-routing kernel v3 for Trainium2 (8 NeuronCores, data-parallel).

Math (u_hat never materialized):
  iter1: c uniform=0.1 -> o1 = 0.1*(sum_n u) @ W_j    (host-precomputed -> q1)
  iter t: Q[:,j] = W_j @ o[j]; logits b = u @ Q; c = softmax_j(b)
          R.T[d,j] = sum_n u[n,d] c[n,j];  o[j] = R[j,:] @ W_j
  out = squash(o3)  (host epilogue)

v3 vs baseline:
  - u loaded ONCE per layout in fp16 (10-bit mantissa): ut [d,n] for logits
    (stationary, FWL), un [n,d] for R (stationary). 8.4MB/core vs 17MB.
  - logits moving operand q is f32r (mixed-dtype MM) or fp16 hi/lo pair.
  - iter-1 chain (depends only on row sums of u) computed on host -> q1.
  - samples processed in pairs to amortize DVE/ACT fixed costs; softmax
    reads logits PSUM directly (no copy).
  - emission is phase-blocked across pairs so PE never waits on DVE.
"""

import os
import sys

import numpy as np

for _p in ("/opt/trn_rl_repo", "/opt/trn_rl_repo/concourse"):
    if _p not in sys.path and os.path.isdir(_p):
        sys.path.insert(0, _p)

import concourse.bass as bass
import concourse.mybir as mybir
import concourse.tile as tile
from concourse import bacc

F32 = mybir.dt.float32
F32R = mybir.dt.float32r
F16 = mybir.dt.float16
AF = mybir.ActivationFunctionType
AX = mybir.AxisListType
ALU = mybir.AluOpType

N_CORES = 8
B_FULL, N, D = 64, 2048, 128
J, DC = 10, 16
JD = J * DC          # 160
NT = N // 128        # 16 chunks of n per sample
B_LOC = B_FULL // N_CORES  # 8 samples per core
NP = B_LOC // 2      # 4 sample pairs
EPS = 1e-7

Q_MODE = os.environ.get("CAPS_Q_MODE", "single")  # "single" (f16 q) | "hilo" (f16 q pair)
WARMUP_MMS = int(os.environ.get("CAPS_WARMUP", "16"))


def _bcast(ap, extra):
    """Append step-0 (broadcast) dims to an AP."""
    return bass.AP(tensor=ap.tensor, offset=ap.offset,
                   ap=list(ap.ap) + [[0, n] for n in extra])


def _bcast_mid(ap, idx, n):
    """Insert a step-0 (broadcast) dim of extent n at position idx (free dims
    count partition as 0)."""
    aps = list(ap.ap)
    aps.insert(idx, [0, n])
    return bass.AP(tensor=ap.tensor, offset=ap.offset, ap=aps)


def build_program(for_sim=False):
    if for_sim:
        nc = bacc.Bacc(None, target_bir_lowering=False, debug=True)
    else:
        nc = bacc.Bacc(None)

    QW = 10 if Q_MODE == "single" else 20
    QDT = F16

    ut_d = nc.declare_dram_parameter("ut", [B_LOC, D, N], F16, isOutput=False)
    un_d = nc.declare_dram_parameter("un", [B_LOC, D, NT, D], F16, isOutput=False)
    q1_d = nc.declare_dram_parameter("q1", [D, B_LOC, QW], QDT, isOutput=False)
    w_d = nc.declare_dram_parameter("w", [D, JD], F32, isOutput=False)
    om_d = nc.declare_dram_parameter("ones_mat", [D, D], F32R, isOutput=False)
    om16_d = nc.declare_dram_parameter("ones16", [D, D], F16, isOutput=False)
    out_d = nc.declare_dram_parameter("out", [1, B_LOC * JD], F32, isOutput=True)

    with tile.TileContext(nc) as tc:
        with (
            tc.tile_pool(name="big", bufs=1) as big,
            tc.tile_pool(name="consts", bufs=1) as consts,
            tc.tile_pool(name="sm", bufs=3) as sm,
            tc.tile_pool(name="chain", bufs=3) as chain,
            tc.tile_pool(name="psumB", bufs=3, space="PSUM") as psumB,
            tc.tile_pool(name="psumR", bufs=2, space="PSUM") as psumR,
            tc.tile_pool(name="psumC", bufs=2, space="PSUM") as psumC,
            tc.tile_pool(name="psumW", bufs=1, space="PSUM") as psumW,
        ):
            w_sb = consts.tile([D, JD], F32)
            ones_r = consts.tile([D, D], F32R)
            q1_sb = consts.tile([D, B_LOC, QW], QDT)
            out_sb = consts.tile([1, B_LOC * JD], F32)
            ones16 = consts.tile([D, D], F16)
            # early tiny consts on the gpsimd ring; mid-kernel consts on
            # scalar; sync carries only the big streams.
            nc.gpsimd.dma_start(out=ones16[:], in_=om16_d[:])
            nc.gpsimd.dma_start(out=q1_sb[:], in_=q1_d[:])
            nc.scalar.dma_start(out=w_sb[:], in_=w_d[:])
            nc.scalar.dma_start(out=ones_r[:], in_=om_d[:])

            w_jd = w_sb[:].rearrange("p (j d) -> p j d", j=J)

            ut = [big.tile([D, NT, D], F16, tag=f"ut{b}", name=f"ut{b}")
                  for b in range(B_LOC)]
            un = [big.tile([D, NT, D], F16, tag=f"un{b}", name=f"un{b}")
                  for b in range(B_LOC)]

            # HAM warmup: dense junk matmuls while the first DMAs land.
            junk = psumW.tile([D, 32], F32, tag="junk", name="junk")
            for _ in range(WARMUP_MMS):
                nc.tensor.matmul(junk[:], ones16[:], ones16[:, 0:32],
                                 start=True, stop=True)

            # big streams on sync, ordered by first consumer
            big_order = ["ut0", "ut1", "un0", "ut2", "un1", "ut3", "un2",
                         "ut4", "un3", "ut5", "un4", "ut6", "un5", "ut7",
                         "un6", "un7"]
            for name in big_order:
                b = int(name[2])
                if name.startswith("ut"):
                    nc.sync.dma_start(
                        out=ut[b][:],
                        in_=ut_d[b, :, :].rearrange("p (t n) -> p t n", t=NT))
                else:
                    nc.sync.dma_start(out=un[b][:], in_=un_d[b])

            def logits(p, q_aps):
                """q_aps: per-sample [D, QW] moving APs.
                hilo: two per-sample [D, NT, 2J] PSUM tiles (hi|lo cols).
                single: one [D, 2, NT, J] pair PSUM tile."""
                if Q_MODE == "single":
                    bp = psumB.tile([D, 2, NT, J], F32, tag="bp")
                    for s in range(2):
                        b = 2 * p + s
                        for t in range(NT):
                            nc.tensor.matmul(bp[:, s, t, :], ut[b][:, t, :],
                                             q_aps[s], start=True, stop=True)
                    return bp
                bps = []
                for s in range(2):
                    b = 2 * p + s
                    bp = psumB.tile([D, NT, 2 * J], F32, tag="bp")
                    for t in range(NT):
                        nc.tensor.matmul(bp[:, t, :], ut[b][:, t, :],
                                         q_aps[s], start=True, stop=True)
                    bps.append(bp)
                return bps

            def softmax(bp):
                """-> c [D, 2, NT, J] fp16."""
                if Q_MODE == "single":
                    bsum = bp[:]           # PSUM AP [D, 2, NT, J]
                    negm = sm.tile([D, 2, NT], F32, tag="negm")
                    nc.vector.reduce_max(negm[:], bsum, axis=AX.X, negate=True)
                else:
                    # ACT-copy each sample's hi|lo PSUM to SBUF, fold on DVE
                    bsum_t = sm.tile([D, 2, NT, J], F32, tag="bsum")
                    for s in range(2):
                        bpc = sm.tile([D, NT, 2 * J], F32, tag=f"bpc{s}")
                        nc.scalar.activation(
                            bpc[:].rearrange("p t j -> p (t j)"),
                            bp[s][:].rearrange("p t j -> p (t j)"), AF.Copy)
                        nc.vector.tensor_add(bsum_t[:, s], bpc[:, :, 0:J],
                                             bpc[:, :, J:2 * J])
                    bsum = bsum_t[:]
                    negm = sm.tile([D, 2, NT], F32, tag="negm")
                    nc.vector.reduce_max(negm[:], bsum, axis=AX.X, negate=True)
                bs = sm.tile([D, 2, NT, J], F32, tag="bs")
                nc.vector.tensor_add(bs[:], bsum, _bcast(negm[:], [J]))
                e = sm.tile([D, 2, NT, J], F16, tag="e")
                nc.scalar.activation(
                    e[:].rearrange("p a t j -> p (a t j)"),
                    bs[:].rearrange("p a t j -> p (a t j)"), AF.Exp)
                z = sm.tile([D, 2, NT], F32, tag="z")
                with nc.allow_low_precision(reason="z sums 10 fp16 probs"):
                    nc.vector.reduce_sum(z[:], e[:], axis=AX.X)
                zr = sm.tile([D, 2, NT], F16, tag="zr")
                with nc.allow_low_precision(reason="zr fp16; z in [1,10]"):
                    nc.vector.reciprocal(zr[:], z[:])
                c = sm.tile([D, 2, NT, J], F16, tag="c")
                nc.vector.tensor_mul(c[:], e[:], _bcast(zr[:], [J]))
                return c

            def r_pass(p, c):
                rt = psumR.tile([D, 2, J], F32, tag="rt")
                for s in range(2):
                    b = 2 * p + s
                    for t in range(NT):
                        nc.tensor.matmul(rt[:, s, :], un[b][:, t, :],
                                         c[:, s, t, :], start=(t == 0),
                                         stop=(t == NT - 1))
                return rt

            def ochain(p, rt, is_last):
                """rt: [D, 2, J] PSUM (R.T per sample). -> q pair or None.
                PSUM->SBUF hops on ACT; elementwise/reduce on GPSIMD (DVE is
                the softmax bottleneck, GPSIMD is otherwise idle)."""
                m1 = chain.tile([D, 2, J, DC], F32R, tag="m1")
                nc.vector.tensor_mul(m1[:], _bcast_mid(w_jd, 1, 2),
                                     _bcast(rt[:], [DC]))
                obc = psumC.tile([D, 2, JD], F32, tag="obc")
                nc.tensor.matmul(obc[:].rearrange("p a f -> p (a f)"),
                                 ones_r[:],
                                 m1[:].rearrange("p a j d -> p (a j d)"),
                                 start=True, stop=True)
                if is_last:
                    sl = out_sb[0:1, 2 * p * JD:(2 * p + 2) * JD]
                    nc.scalar.activation(
                        sl, obc[0:1, :, :].rearrange("p a f -> p (a f)"),
                        AF.Copy)
                    nc.sync.dma_start(
                        out=out_d[0, 2 * p * JD:(2 * p + 2) * JD].unsqueeze(0),
                        in_=sl)
                    return None
                qw = chain.tile([D, 2, J, DC], F32, tag="qw")
                nc.vector.tensor_mul(
                    qw[:], _bcast_mid(w_jd, 1, 2),
                    obc[:].rearrange("p a (j d) -> p a j d", j=J))
                q = chain.tile([D, 2, J], F16, tag="q")
                with nc.allow_low_precision(reason="q fp16 feeds fp16 MM"):
                    nc.vector.reduce_sum(q[:], qw[:], axis=AX.X)
                return [q[:, 0, :], q[:, 1, :]]

            # Interleaved emission: PE executes in emission order, so order
            # blocks by data arrival (ut0..ut7 then un0..un7) and keep
            # un7-dependent work late while independent iter-3 work fills in.
            ORDER = ["L2_0", "R2_0", "L2_1", "R2_1", "O2_0", "L3_0",
                     "L2_2", "O2_1", "R2_2", "R3_0", "L3_1", "L2_3",
                     "O2_2", "O3_0", "R2_3", "L3_2", "R3_1", "O2_3",
                     "O3_1", "L3_3", "R3_2", "O3_2", "R3_3", "O3_3"]
            q_cur = {p: [q1_sb[:, 2 * p, :], q1_sb[:, 2 * p + 1, :]]
                     for p in range(NP)}
            cs, rts = {}, {}
            for blk in ORDER:
                kind, p = blk.split("_")
                p = int(p)
                if kind in ("L2", "L3"):
                    cs[p] = softmax(logits(p, q_cur[p]))
                elif kind in ("R2", "R3"):
                    rts[p] = r_pass(p, cs[p])
                elif kind == "O2":
                    q_cur[p] = ochain(p, rts[p], False)
                else:
                    ochain(p, rts[p], True)

    nc.compile()
    return nc


def _f32r(x):
    xi = np.ascontiguousarray(x, np.float32).view(np.uint32).astype(np.int64)
    bias = ((xi >> 12) & 1) + (1 << 11) - 1
    return (((xi + bias) >> 12) << 12).astype(np.uint32).view(np.float32)


def _squash(o):
    s2 = (o ** 2).sum(-1, keepdims=True)
    return o * s2 / ((1.0 + s2) * np.sqrt(s2 + EPS))


def host_inputs(u_core, W):
    """Per-core host prep: u_core [B_LOC, N, D] f32, W [D, JD] f32."""
    us = np.ascontiguousarray(u_core, np.float32)
    ut = np.ascontiguousarray(us.transpose(0, 2, 1)).astype(np.float16)
    un = np.ascontiguousarray(
        us.reshape(B_LOC, NT, D, D).transpose(0, 2, 1, 3)).astype(np.float16)
    # iter-1 chain on host: r1 = 0.1*sum_n u -> o1 -> q1
    Wr = W.reshape(D, J, DC)
    r1 = 0.1 * us.sum(axis=1)                         # [B_LOC, D]
    m1 = _f32r(Wr[None] * r1[:, :, None, None])       # [B_LOC, D, J, DC]
    o1 = m1.sum(axis=1)                               # [B_LOC, J, DC]
    q1 = (Wr[None] * o1[:, None, :, :]).sum(-1)       # [B_LOC, D, J]
    if Q_MODE == "single":
        q1_np = np.ascontiguousarray(q1.astype(np.float16).transpose(1, 0, 2))
    else:
        q1h = q1.astype(np.float16)
        q1l = (q1 - q1h.astype(np.float32)).astype(np.float16)
        q1_np = np.ascontiguousarray(
            np.concatenate([q1h, q1l], axis=-1).transpose(1, 0, 2))
    return {
        "ut": ut,
        "un": un,
        "q1": q1_np,
        "w": np.ascontiguousarray(W, np.float32),
        "ones_mat": np.ones((D, D), np.float32),
        "ones16": np.ones((D, D), np.float16),
    }


_NC = None


def _get_nc():
    global _NC
    if _NC is None:
        _NC = build_program()
    return _NC


def run_sharded(u_vecs: np.ndarray, W: np.ndarray, **kw):
    """Shard over 8 cores, run, return (full_output, BassKernelResults)."""
    from concourse.bass_utils import run_bass_kernel_spmd

    u_vecs = np.ascontiguousarray(u_vecs, dtype=np.float32)
    W = np.ascontiguousarray(W, dtype=np.float32)
    assert u_vecs.shape == (B_FULL, N, D) and W.shape == (D, JD)

    nc = _get_nc()
    in_maps = [host_inputs(u_vecs[k * B_LOC:(k + 1) * B_LOC], W)
               for k in range(N_CORES)]
    res = run_bass_kernel_spmd(nc, in_maps, core_ids=list(range(N_CORES)), **kw)
    o3 = np.concatenate(
        [res.results[k]["out"].reshape(B_LOC, JD) for k in range(N_CORES)],
        axis=0)
    out = _squash(o3.reshape(B_FULL, J, DC).astype(np.float32))
    return out.astype(np.float32), res


def kernel(u_vecs: np.ndarray, W: np.ndarray) -> np.ndarray:
    out, _ = run_sharded(u_vecs, W)
    return out


# revision 9
# speedup vs baseline: 1.1123x; 1.0109x over previous
"""Capsule-routing kernel v3 for Trainium2 (8 NeuronCores, data-parallel).

Math (u_hat never materialized):
  iter1: c uniform=0.1 -> o1 = 0.1*(sum_n u) @ W_j    (host-precomputed -> q1)
  iter t: Q[:,j] = W_j @ o[j]; logits b = u @ Q; c = softmax_j(b)
          R.T[d,j] = sum_n u[n,d] c[n,j];  o[j] = R[j,:] @ W_j
  out = squash(o3)  (host epilogue)

v3 vs baseline:
  - u loaded ONCE per layout in fp16 (10-bit mantissa): ut [d,n] for logits
    (stationary, FWL), un [n,d] for R (stationary). 8.4MB/core vs 17MB.
  - logits moving operand q is f32r (mixed-dtype MM) or fp16 hi/lo pair.
  - iter-1 chain (depends only on row sums of u) computed on host -> q1.
  - samples processed in pairs to amortize DVE/ACT fixed costs; softmax
    reads logits PSUM directly (no copy).
  - emission is phase-blocked across pairs so PE never waits on DVE.
"""

import os
import sys

import numpy as np

for _p in ("/opt/trn_rl_repo", "/opt/trn_rl_repo/concourse"):
    if _p not in sys.path and os.path.isdir(_p):
        sys.path.insert(0, _p)

import concourse.bass as bass
import concourse.mybir as mybir
import concourse.tile as tile
from concourse import bacc

F32 = mybir.dt.float32
F32R = mybir.dt.float32r
F16 = mybir.dt.float16
AF = mybir.ActivationFunctionType
AX = mybir.AxisListType
ALU = mybir.AluOpType

N_CORES = 8
B_FULL, N, D = 64, 2048, 128
J, DC = 10, 16
JD = J * DC          # 160
NT = N // 128        # 16 chunks of n per sample
B_LOC = B_FULL // N_CORES  # 8 samples per core
NP = B_LOC // 2      # 4 sample pairs
EPS = 1e-7

Q_MODE = os.environ.get("CAPS_Q_MODE", "single")  # "single" (f16 q) | "hilo" (f16 q pair)
WARMUP_MMS = int(os.environ.get("CAPS_WARMUP", "0"))


def _bcast(ap, extra):
    """Append step-0 (broadcast) dims to an AP."""
    return bass.AP(tensor=ap.tensor, offset=ap.offset,
                   ap=list(ap.ap) + [[0, n] for n in extra])


def _bcast_mid(ap, idx, n):
    """Insert a step-0 (broadcast) dim of extent n at position idx (free dims
    count partition as 0)."""
    aps = list(ap.ap)
    aps.insert(idx, [0, n])
    return bass.AP(tensor=ap.tensor, offset=ap.offset, ap=aps)


def build_program(for_sim=False):
    if for_sim:
        nc = bacc.Bacc(None, target_bir_lowering=False, debug=True)
    else:
        nc = bacc.Bacc(None)

    QW = 10 if Q_MODE == "single" else 20
    QDT = F16

    ut_d = nc.declare_dram_parameter("ut", [B_LOC, D, N], F16, isOutput=False)
    un_d = nc.declare_dram_parameter("un", [B_LOC, D, NT, D], F16, isOutput=False)
    q1_d = nc.declare_dram_parameter("q1", [D, B_LOC, QW], QDT, isOutput=False)
    w_d = nc.declare_dram_parameter("w", [D, JD], F32, isOutput=False)
    om_d = nc.declare_dram_parameter("ones_mat", [D, D], F32R, isOutput=False)
    out_d = nc.declare_dram_parameter("out", [1, B_LOC * JD], F32, isOutput=True)

    with tile.TileContext(nc) as tc:
        with (
            tc.tile_pool(name="big", bufs=1) as big,
            tc.tile_pool(name="consts", bufs=1) as consts,
            tc.tile_pool(name="sm", bufs=3) as sm,
            tc.tile_pool(name="chain", bufs=3) as chain,
            tc.tile_pool(name="psumB", bufs=4, space="PSUM") as psumB,
            tc.tile_pool(name="psumR", bufs=2, space="PSUM") as psumR,
            tc.tile_pool(name="psumC", bufs=2, space="PSUM") as psumC,
        ):
            w_sb = consts.tile([D, JD], F32)
            ones_r = consts.tile([D, D], F32R)
            q1_sb = consts.tile([D, B_LOC, QW], QDT)
            out_sb = consts.tile([1, B_LOC * JD], F32)
            # early tiny const on the gpsimd ring; mid-kernel consts on
            # scalar; sync carries only the big streams.
            nc.gpsimd.dma_start(out=q1_sb[:], in_=q1_d[:])
            nc.scalar.dma_start(out=w_sb[:], in_=w_d[:])
            nc.scalar.dma_start(out=ones_r[:], in_=om_d[:])

            w_jd = w_sb[:].rearrange("p (j d) -> p j d", j=J)

            ut = [big.tile([D, NT, D], F16, tag=f"ut{b}", name=f"ut{b}")
                  for b in range(B_LOC)]
            un = [big.tile([D, NT, D], F16, tag=f"un{b}", name=f"un{b}")
                  for b in range(B_LOC)]

            # big streams on sync, ordered by first consumer
            big_order = ["ut0", "ut1", "un0", "ut2", "un1", "ut3", "un2",
                         "ut4", "un3", "ut5", "un4", "ut6", "un5", "ut7",
                         "un6", "un7"]
            for name in big_order:
                b = int(name[2])
                if name.startswith("ut"):
                    nc.sync.dma_start(
                        out=ut[b][:],
                        in_=ut_d[b, :, :].rearrange("p (t n) -> p t n", t=NT))
                else:
                    nc.sync.dma_start(out=un[b][:], in_=un_d[b])

            def logits(p, q_aps):
                """q_aps: per-sample [D, QW] moving APs.
                hilo: two per-sample [D, NT, 2J] PSUM tiles (hi|lo cols).
                single: one [D, 2, NT, J] pair PSUM tile."""
                if Q_MODE == "single":
                    bp = psumB.tile([D, 2, NT, J], F32, tag="bp")
                    for s in range(2):
                        b = 2 * p + s
                        for t in range(NT):
                            nc.tensor.matmul(bp[:, s, t, :], ut[b][:, t, :],
                                             q_aps[s], start=True, stop=True)
                    return bp
                bps = []
                for s in range(2):
                    b = 2 * p + s
                    bp = psumB.tile([D, NT, 2 * J], F32, tag="bp")
                    for t in range(NT):
                        nc.tensor.matmul(bp[:, t, :], ut[b][:, t, :],
                                         q_aps[s], start=True, stop=True)
                    bps.append(bp)
                return bps

            def softmax(bp):
                """-> c [D, 2, NT, J] fp16."""
                if Q_MODE == "single":
                    bsum = bp[:]           # PSUM AP [D, 2, NT, J]
                    negm = sm.tile([D, 2, NT], F32, tag="negm")
                    nc.vector.reduce_max(negm[:], bsum, axis=AX.X, negate=True)
                else:
                    # ACT-copy each sample's hi|lo PSUM to SBUF, fold on DVE
                    bsum_t = sm.tile([D, 2, NT, J], F32, tag="bsum")
                    for s in range(2):
                        bpc = sm.tile([D, NT, 2 * J], F32, tag=f"bpc{s}")
                        nc.scalar.activation(
                            bpc[:].rearrange("p t j -> p (t j)"),
                            bp[s][:].rearrange("p t j -> p (t j)"), AF.Copy)
                        nc.vector.tensor_add(bsum_t[:, s], bpc[:, :, 0:J],
                                             bpc[:, :, J:2 * J])
                    bsum = bsum_t[:]
                    negm = sm.tile([D, 2, NT], F32, tag="negm")
                    nc.vector.reduce_max(negm[:], bsum, axis=AX.X, negate=True)
                bs = sm.tile([D, 2, NT, J], F32, tag="bs")
                nc.vector.tensor_add(bs[:], bsum, _bcast(negm[:], [J]))
                e = sm.tile([D, 2, NT, J], F16, tag="e")
                nc.scalar.activation(
                    e[:].rearrange("p a t j -> p (a t j)"),
                    bs[:].rearrange("p a t j -> p (a t j)"), AF.Exp)
                z = sm.tile([D, 2, NT], F32, tag="z")
                with nc.allow_low_precision(reason="z sums 10 fp16 probs"):
                    nc.vector.reduce_sum(z[:], e[:], axis=AX.X)
                zr = sm.tile([D, 2, NT], F16, tag="zr")
                with nc.allow_low_precision(reason="zr fp16; z in [1,10]"):
                    nc.vector.reciprocal(zr[:], z[:])
                c = sm.tile([D, 2, NT, J], F16, tag="c")
                nc.vector.tensor_mul(c[:], e[:], _bcast(zr[:], [J]))
                return c

            def r_pass(p, c):
                rt = psumR.tile([D, 2, J], F32, tag="rt")
                for s in range(2):
                    b = 2 * p + s
                    for t in range(NT):
                        nc.tensor.matmul(rt[:, s, :], un[b][:, t, :],
                                         c[:, s, t, :], start=(t == 0),
                                         stop=(t == NT - 1))
                return rt

            def ochain(p, rt, is_last):
                """rt: [D, 2, J] PSUM (R.T per sample). -> q pair or None.
                PSUM->SBUF hops on ACT; elementwise/reduce on GPSIMD (DVE is
                the softmax bottleneck, GPSIMD is otherwise idle)."""
                m1 = chain.tile([D, 2, J, DC], F32R, tag="m1")
                nc.vector.tensor_mul(m1[:], _bcast_mid(w_jd, 1, 2),
                                     _bcast(rt[:], [DC]))
                obc = psumC.tile([D, 2, JD], F32, tag="obc")
                nc.tensor.matmul(obc[:].rearrange("p a f -> p (a f)"),
                                 ones_r[:],
                                 m1[:].rearrange("p a j d -> p (a j d)"),
                                 start=True, stop=True)
                if is_last:
                    sl = out_sb[0:1, 2 * p * JD:(2 * p + 2) * JD]
                    nc.scalar.activation(
                        sl, obc[0:1, :, :].rearrange("p a f -> p (a f)"),
                        AF.Copy)
                    nc.sync.dma_start(
                        out=out_d[0, 2 * p * JD:(2 * p + 2) * JD].unsqueeze(0),
                        in_=sl)
                    return None
                qw = chain.tile([D, 2, J, DC], F32, tag="qw")
                nc.vector.tensor_mul(
                    qw[:], _bcast_mid(w_jd, 1, 2),
                    obc[:].rearrange("p a (j d) -> p a j d", j=J))
                q = chain.tile([D, 2, J], F16, tag="q")
                with nc.allow_low_precision(reason="q fp16 feeds fp16 MM"):
                    nc.vector.reduce_sum(q[:], qw[:], axis=AX.X)
                return [q[:, 0, :], q[:, 1, :]]

            # Interleaved emission: PE executes in emission order, so order
            # blocks by data arrival (ut0..ut7 then un0..un7) and keep
            # un7-dependent work late while independent iter-3 work fills in.
            ORDER = ["L2_0", "R2_0", "L2_1", "R2_1", "O2_0", "L3_0",
                     "L2_2", "O2_1", "R2_2", "R3_0", "L3_1", "L2_3",
                     "O2_2", "O3_0", "R2_3", "L3_2", "R3_1", "O2_3",
                     "O3_1", "L3_3", "R3_2", "O3_2", "R3_3", "O3_3"]
            q_cur = {p: [q1_sb[:, 2 * p, :], q1_sb[:, 2 * p + 1, :]]
                     for p in range(NP)}
            cs, rts = {}, {}
            for blk in ORDER:
                kind, p = blk.split("_")
                p = int(p)
                if kind in ("L2", "L3"):
                    cs[p] = softmax(logits(p, q_cur[p]))
                elif kind in ("R2", "R3"):
                    rts[p] = r_pass(p, cs[p])
                elif kind == "O2":
                    q_cur[p] = ochain(p, rts[p], False)
                else:
                    ochain(p, rts[p], True)

    nc.compile()
    return nc


def _f32r(x):
    xi = np.ascontiguousarray(x, np.float32).view(np.uint32).astype(np.int64)
    bias = ((xi >> 12) & 1) + (1 << 11) - 1
    return (((xi + bias) >> 12) << 12).astype(np.uint32).view(np.float32)


def _squash(o):
    s2 = (o ** 2).sum(-1, keepdims=True)
    return o * s2 / ((1.0 + s2) * np.sqrt(s2 + EPS))


def host_inputs(u_core, W):
    """Per-core host prep: u_core [B_LOC, N, D] f32, W [D, JD] f32."""
    us = np.ascontiguousarray(u_core, np.float32)
    ut = np.ascontiguousarray(us.transpose(0, 2, 1)).astype(np.float16)
    un = np.ascontiguousarray(
        us.reshape(B_LOC, NT, D, D).transpose(0, 2, 1, 3)).astype(np.float16)
    # iter-1 chain on host: r1 = 0.1*sum_n u -> o1 -> q1
    Wr = W.reshape(D, J, DC)
    r1 = 0.1 * us.sum(axis=1)                         # [B_LOC, D]
    m1 = _f32r(Wr[None] * r1[:, :, None, None])       # [B_LOC, D, J, DC]
    o1 = m1.sum(axis=1)                               # [B_LOC, J, DC]
    q1 = (Wr[None] * o1[:, None, :, :]).sum(-1)       # [B_LOC, D, J]
    if Q_MODE == "single":
        q1_np = np.ascontiguousarray(q1.astype(np.float16).transpose(1, 0, 2))
    else:
        q1h = q1.astype(np.float16)
        q1l = (q1 - q1h.astype(np.float32)).astype(np.float16)
        q1_np = np.ascontiguousarray(
            np.concatenate([q1h, q1l], axis=-1).transpose(1, 0, 2))
    return {
        "ut": ut,
        "un": un,
        "q1": q1_np,
        "w": np.ascontiguousarray(W, np.float32),
        "ones_mat": np.ones((D, D), np.float32),
    }


_NC = None


def _get_nc():
    global _NC
    if _NC is None:
        _NC = build_program()
    return _NC


def run_sharded(u_vecs: np.ndarray, W: np.ndarray, **kw):
    """Shard over 8 cores, run, return (full_output, BassKernelResults)."""
    from concourse.bass_utils import run_bass_kernel_spmd

    u_vecs = np.ascontiguousarray(u_vecs, dtype=np.float32)
    W = np.ascontiguousarray(W, dtype=np.float32)
    assert u_vecs.shape == (B_FULL, N, D) and W.shape == (D, JD)

    nc = _get_nc()
    in_maps = [host_inputs(u_vecs[k * B_LOC:(k + 1) * B_LOC], W)
               for k in range(N_CORES)]
    res = run_bass_kernel_spmd(nc, in_maps, core_ids=list(range(N_CORES)), **kw)
    o3 = np.concatenate(
        [res.results[k]["out"].reshape(B_LOC, JD) for k in range(N_CORES)],
        axis=0)
    out = _squash(o3.reshape(B_FULL, J, DC).astype(np.float32))
    return out.astype(np.float32), res


def kernel(u_vecs: np.ndarray, W: np.ndarray) -> np.ndarray:
    out, _ = run_sharded(u_vecs, W)
    return out
